# revision 11
# baseline (speedup 1.0000x reference)
"""Trainium2 Bass kernel for MDPPInitEmbedding (retrieval_knn).

Math: the reference network folds exactly to
    out[b,j,:] = locs[b,j,:] @ A + min_dist[b,j] * v + c
with A = W_node @ W_out[:E], v = W_dist @ W_out[E:],
c = b_node @ W_out[:E] + b_dist @ W_out[E:] + b_out.

min_dist[b,j] = sqrt(max(0, min_{i in probes} d2[i,j])) where
    d2[i,j] = sq_i + sq_j - 2*x_i.x_j
is computed on the PE as a K=4 matmul:
    stationary (per j): [-2*x_j0, -2*x_j1, 1, sq_j]
    moving  (per cand): [ x_i0,    x_i1,  a_i, 1 ]  (a_i = sq_i; pads use 1e30)
so PSUM holds d2 directly (no cancellation on the small values).

Probes are compacted host-side (padded to a multiple of 256) so the device
only scans real candidates. Per j-block of 128, the masked min over
candidates runs as: scalar-engine copy of the second PSUM half to SBUF,
DVE tensor_tensor(min) pairing the PSUM half with the SBUF half (2 cands/
cycle, bf16 out), then one DVE tensor_scalar with a fused min accum_out
(4x bf16 read) -> [128,1] block minima. Sharding: data-parallel over batch
B=16, 2 batches per NeuronCore across 8 cores.
"""

import numpy as np

import concourse.bass as bass
import concourse.bacc as bacc
import concourse.tile as tile
from concourse import mybir
from concourse.bass_utils import run_bass_kernel_spmd

B, N, E = 16, 2048, 256
NCORES = 8
NB = B // NCORES          # batches per core
NBLK = N // 128           # j-blocks per batch
JB = 128
F32 = mybir.dt.float32
BF16 = mybir.dt.bfloat16
BIG = 1.0e30

_PROG_CACHE = {}


def _build_program(P, reps=1, mode="full"):
    """Bass program for one core: NB batches, P padded candidates each."""
    assert P % 256 == 0
    half = P // 2
    nc = bacc.Bacc("TRN2", target_bir_lowering=False, debug=False,
                   num_devices=NCORES)

    rhs_d = nc.dram_tensor("rhs", [NB, 4, P], F32, kind="ExternalInput").ap()
    wj_d = nc.dram_tensor("wj", [NB, 4, N], F32, kind="ExternalInput").ap()
    xu_d = nc.dram_tensor("xu", [NB, 4, N], F32, kind="ExternalInput").ap()
    w4_d = nc.dram_tensor("w4", [4, E], F32, kind="ExternalInput").ap()
    eye_d = nc.dram_tensor("eye", [128, 128], F32, kind="ExternalInput").ap()
    out_d = nc.dram_tensor("out", [NB, N, E], F32, kind="ExternalOutput").ap()

    mn = mybir.AluOpType.min

    with tile.TileContext(nc) as tc:
        with (
            tc.tile_pool(name="const", bufs=1) as const_pool,
            tc.tile_pool(name="inputs", bufs=2) as in_pool,
            tc.tile_pool(name="halves", bufs=3) as half_pool,
            tc.tile_pool(name="trash", bufs=2) as trash_pool,
            tc.tile_pool(name="md", bufs=2) as md_pool,
            tc.tile_pool(name="ostage", bufs=4) as ostage_pool,
            tc.tile_pool(name="dps", bufs=2, space="PSUM") as dist_psum,
            tc.tile_pool(name="ops", bufs=2, space="PSUM") as out_psum,
        ):
            w4 = const_pool.tile([4, E], F32)
            nc.sync.dma_start(w4[:], w4_d[:])
            eye = const_pool.tile([128, 128], F32)
            nc.sync.dma_start(eye[:], eye_d[:])

            for b in [b for _ in range(reps) for b in range(NB)]:
                rhs = in_pool.tile([4, P], F32, tag="rhs")
                nc.sync.dma_start(rhs[:], rhs_d[b])
                wj = in_pool.tile([4, N], F32, tag="wj")
                nc.sync.dma_start(wj[:], wj_d[b])
                u = in_pool.tile([4, N], F32, tag="u")
                nc.sync.dma_start(u[:], xu_d[b])

                md2 = md_pool.tile([128, NBLK], F32, tag="md2")

                # distance + masked-min phase
                for blk in range(NBLK):
                    ps = dist_psum.tile([128, P], F32, tag="d")
                    for c0 in range(0, P, 512):
                        w = min(512, P - c0)
                        nc.tensor.matmul(
                            ps[:, c0:c0 + w],
                            wj[:, blk * JB:(blk + 1) * JB],
                            rhs[:, c0:c0 + w],
                            start=True, stop=True,
                        )
                    if mode == "mm":
                        continue
                    sb = half_pool.tile([128, half], F32, tag="h")
                    nc.scalar.copy(sb[:], ps[:, half:P])
                    tr = trash_pool.tile([128, half], BF16, tag="t")
                    nc.vector.tensor_tensor(tr[:], ps[:, 0:half], sb[:], op=mn)
                    tr2 = trash_pool.tile([128, half], BF16, tag="t2")
                    nc.vector.tensor_scalar(
                        out=tr2[:], in0=tr[:], scalar1=BIG, scalar2=None,
                        op0=mn, op1=mn, accum_out=md2[:, blk:blk + 1],
                    )

                if mode in ("mm", "dist"):
                    continue
                # md2 -> md row in U
                md2c = md_pool.tile([128, NBLK], F32, tag="md2c")
                nc.vector.tensor_scalar_max(md2c[:], md2[:], 0.0)
                mds = md_pool.tile([128, NBLK], F32, tag="mds")
                nc.scalar.sqrt(mds[:], md2c[:])
                mdt_ps = out_psum.tile([NBLK, 128], F32, tag="o")
                nc.tensor.transpose(mdt_ps[:], mds[:], eye[:])
                mdt = md_pool.tile([NBLK, 128], F32, tag="mdt")
                nc.scalar.copy(mdt[:], mdt_ps[:])
                nc.sync.dma_start(u[2:3, :], mdt[:])

                # output phase: out[j,:] = U[:,j].T @ W4, two blocks per bank
                for g in range(NBLK // 2):
                    ops = out_psum.tile([128, 2 * E], F32, tag="o")
                    for r in range(2):
                        blk = 2 * g + r
                        nc.tensor.matmul(
                            ops[:, r * E:(r + 1) * E],
                            u[:, blk * JB:(blk + 1) * JB],
                            w4[:],
                            start=True, stop=True,
                        )
                    stage = ostage_pool.tile([128, 2 * E], F32, tag="s")
                    nc.scalar.copy(stage[:], ops[:])
                    for r in range(2):
                        blk = 2 * g + r
                        nc.sync.dma_start(
                            out_d[b, blk * JB:(blk + 1) * JB, :],
                            stage[:, r * E:(r + 1) * E],
                        )
    nc.compile()
    return nc


def _prepare_inputs(locs, probe, W_node, b_node, W_dist, b_dist, W_out, b_out):
    """Fold weights and build per-core input maps."""
    locs = np.asarray(locs, dtype=np.float32)
    probe = np.asarray(probe).astype(bool)

    Wn = np.asarray(W_node, dtype=np.float64)
    bn = np.asarray(b_node, dtype=np.float64)
    Wd = np.asarray(W_dist, dtype=np.float64)
    bd = np.asarray(b_dist, dtype=np.float64)
    Wo = np.asarray(W_out, dtype=np.float64)
    bo = np.asarray(b_out, dtype=np.float64)

    A = Wn @ Wo[:E]                      # [2,E]
    v = Wd @ Wo[E:]                      # [1,E]
    c = bn @ Wo[:E] + bd @ Wo[E:] + bo   # [E]
    w4 = np.stack([A[0], A[1], v[0], c], axis=0).astype(np.float32)

    counts = probe.sum(axis=1)
    P = int(max(512, -(-int(counts.max()) // 256) * 256))

    x0 = locs[:, :, 0]
    x1 = locs[:, :, 1]
    sq = x0 * x0 + x1 * x1               # fp32

    ones = np.ones((N,), dtype=np.float32)
    zeros = np.zeros((N,), dtype=np.float32)

    in_maps = []
    for core in range(NCORES):
        bsl = slice(core * NB, (core + 1) * NB)
        rhs = np.zeros((NB, 4, P), dtype=np.float32)
        wjt = np.zeros((NB, 4, N), dtype=np.float32)
        xut = np.zeros((NB, 4, N), dtype=np.float32)
        for k, b in enumerate(range(bsl.start, bsl.stop)):
            idx = np.nonzero(probe[b])[0]
            pb = len(idx)
            rhs[k, 0, :pb] = x0[b, idx]
            rhs[k, 1, :pb] = x1[b, idx]
            rhs[k, 2, :pb] = sq[b, idx]
            rhs[k, 2, pb:] = BIG
            rhs[k, 3, :pb] = 1.0
            wjt[k] = np.stack([-2.0 * x0[b], -2.0 * x1[b], ones, sq[b]], axis=0)
            xut[k] = np.stack([x0[b], x1[b], zeros, ones], axis=0)
        in_maps.append({
            "rhs": rhs,
            "wj": wjt,
            "xu": xut,
            "w4": w4,
            "eye": np.eye(128, dtype=np.float32),
        })
    return P, in_maps


def _run(inputs, trace=False):
    P, in_maps = _prepare_inputs(**inputs)
    if P not in _PROG_CACHE:
        _PROG_CACHE[P] = _build_program(P)
    nc = _PROG_CACHE[P]
    res = run_bass_kernel_spmd(nc, in_maps, list(range(NCORES)), trace=trace)
    out = np.concatenate([np.asarray(res.results[i]["out"]) for i in range(NCORES)],
                         axis=0)
    return out.reshape(B, N, E).astype(np.float32), res


def kernel(**inputs):
    out, _ = _run(inputs, trace=False)
    return out


def run_traced(inputs):
    return _run(inputs, trace=True)
